# revision 24
# baseline (speedup 1.0000x reference)
"""Causal MHA (B=4, S=4096, D=64, scale=1/sqrt(S)) on 8 trn2 NeuronCores.

Strategy (identical SPMD program on all 8 cores; per-core data differs):
  - scale = 1/sqrt(4096) = 1/64 (reference scales by sqrt of SEQ length).
  - scoresT layout [k, q]: softmax denominator comes free from a ones column
    in the V stationary (row 64 of the AV output); AV needs no transposes.
  - Each core: one batch b = c%4, eight query slots of 256 rows. Slot m
    (1..8) covers query block j_m = 2m-1-(c//4) and iterates 4m k-tiles of
    128 keys (sorted-descending capacity pairing keeps the program uniform).
  - QK: scoresT[k,q] += KT_tile.T @ QT, contraction d=64; adjacent k-tiles
    packed into partition halves run concurrently in disjoint PE row groups.
  - exp is SPLIT across two engines:
      * ScalarE ACTIVATE(Exp, scale=1/64) for most chunks.
      * DVE polynomial exp for the first chunk of big slots:
        y = ((s*(a/128)+b)^2 + c)^2  ~ exp(s/64), 4 DVE ops
        (TS fused mul+add from PSUM, TT square, TS add, TT square).
  - Chunks of 6 or 4 k-tiles; causal/padding masks multiply only the last
    4 k-tiles of each slot, as ONE batched DVE multiply (mask columns are
    host-permuted to match the PSUM bank permutation).
  - AV: outT[d,q] += V_tile(+ones col).T @ expT, fp32 PSUM accumulation.
    AV for the DVE chunk is issued LAST in its slot (accumulation is
    commutative; start/stop flags follow issue order) so the DVE has the
    whole slot duration to finish.
  - Flat software pipeline across all slots: AV work is deferred by LAG
    chunks behind QK/exp emission so the PE never stalls on a fresh exp.
  - Input DMAs are split across the two HWDGE rings (sync + scalar).
  - Output per core: OT [65, 2048] fp32; host divides and scatters.
"""

import sys

sys.path.insert(0, "/opt/trn_rl_repo")

import numpy as np
import ml_dtypes

B, S, D = 4, 4096, 64
NCORES = 8
NSLOTS = 8          # query slots per core, 256 queries each
QS = 256            # queries per slot
KT_TILE = 128       # keys per k-tile
NKT = S // KT_TILE  # 32 k-tiles per batch
BF16 = ml_dtypes.bfloat16

# DVE polynomial exp coefficients: ((s*(PA/128)+PB)^2 + PC)^2 ~ exp(s/64)
PA = 0.7019593264806567
PB = 0.7301021701569483
PC = 0.4682593781298645

_COMPILED = None

LDW_OPT = False
_CACHE_BUST = 5  # sizes the ladder tile; bump to force a NEFF recompile

PERM4 = [0, 2, 1, 3]
PERM6 = [0, 2, 4, 1, 3, 5]


def _chunk_plan(m):
    """Per-slot list of (csize, kind); kind in {'act','dve','mask'}.
    Mixed 6/4-tile chunks (6-tile amortizes the ACT pipe-fill overhead);
    the last chunk is always 4 tiles and masked. The first chunk of slots
    m>=5 goes to the DVE polynomial path — only there does the ~4us serial
    DVE chain hide under the slot's ACT work. Total tiles = 4m."""
    plan = []
    for ci in range(m):
        if ci == m - 1:
            plan.append((4, "mask"))
        elif ci == 1 and m >= 5:
            plan.append((4, "dve"))
        else:
            plan.append((4, "act"))
    return plan


def _build_program():
    import concourse.bacc as bacc
    import concourse.tile as tile
    import concourse.mybir as mybir
    import concourse.bass_utils as _bu

    if LDW_OPT and not getattr(_bu, "_ldw_opt_patched", False):
        _orig_run_command = _bu.run_command

        def _run_command_ldw(argv, **kw):
            argv = [
                "--enable-ldw-opt=true" if a == "--enable-ldw-opt=false" else a
                for a in argv
            ]
            return _orig_run_command(argv, **kw)

        _bu.run_command = _run_command_ldw
        _bu._ldw_opt_patched = True

    F32 = mybir.dt.float32
    MBF16 = mybir.dt.bfloat16
    EXPF = mybir.ActivationFunctionType.Exp
    MUL = mybir.AluOpType.mult
    ADD = mybir.AluOpType.add

    nc = bacc.Bacc("TRN2", target_bir_lowering=False, debug=False, num_devices=NCORES)

    ktp = nc.dram_tensor("ktp", [128, 16 * 128], MBF16, kind="ExternalInput").ap()
    qtd = nc.dram_tensor("qtd", [128, NSLOTS * QS], MBF16, kind="ExternalInput").ap()
    va = nc.dram_tensor("va", [128, NKT * 65], MBF16, kind="ExternalInput").ap()
    mask = nc.dram_tensor("mask", [128, 4 * QS], MBF16, kind="ExternalInput").ap()
    ot = nc.dram_tensor("ot", [65, NSLOTS * QS], F32, kind="ExternalOutput").ap()

    LAG = 2  # AV chunks deferred behind QK/exp emission

    with tile.TileContext(nc) as tc:
        with (
            tc.tile_pool(name="ins", bufs=1) as ins,
            tc.tile_pool(name="work", bufs=3) as work,
            tc.tile_pool(name="outs", bufs=2) as outs,
            tc.tile_pool(name="ps", bufs=1, space="PSUM") as ps,
            tc.tile_pool(name="pso", bufs=1, space="PSUM") as pso,
        ):
            ktp_sb = ins.tile([128, 16 * 128], MBF16)
            qtd_sb = ins.tile([128, NSLOTS * QS], MBF16)
            va_sb = ins.tile([128, NKT * 65], MBF16)
            mask_sb = ins.tile([128, 4 * QS], MBF16)
            # input DMAs split across the two HWDGE rings, each ordered by
            # consumption (slot 8 first)
            # input DMAs across three rings. The scalar ring carries ONLY
            # ktp_lo: every dma_start occupies its queue ~0.7us, and the
            # scalar queue must get to ACT_TABLE_LOAD + the first ACTIVATE
            # as early as possible (it gates the whole ACT stream).
            nc.sync.dma_start(out=qtd_sb[:, 7 * QS :], in_=qtd[:, 7 * QS :])
            nc.scalar.dma_start(out=ktp_sb[:, :1024], in_=ktp[:, :1024])
            nc.sync.dma_start(out=va_sb[:, :1040], in_=va[:, :1040])
            nc.gpsimd.dma_start(out=ktp_sb[:, 1024:], in_=ktp[:, 1024:])
            nc.sync.dma_start(out=qtd_sb[:, 4 * QS : 7 * QS], in_=qtd[:, 4 * QS : 7 * QS])
            nc.gpsimd.dma_start(out=mask_sb, in_=mask)
            nc.sync.dma_start(out=va_sb[:, 1040:], in_=va[:, 1040:])
            nc.gpsimd.dma_start(out=qtd_sb[:, : 4 * QS], in_=qtd[:, : 4 * QS])

            # wait ladders: absorb one DMA-queue sem per tiny op so real
            # instructions never need >1 sync wait (HW allows 1 per inst).
            # Entries are STAGED at the point each tensor is first needed —
            # a single up-front ladder would make the first QK (strict PE
            # FIFO) wait for the LAST input DMA's ~2us write receipt.
            # The ladder tile shares the avout tag so it doesn't cost its own
            # PSUM bank (nothing reads it; the bank is recycled by slot 7).
            lad_ps = pso.tile([65, QS], F32, tag="avout", bufs=2)
            lad_dve = work.tile([1, 8], MBF16, tag="lad_dve", bufs=1)
            _lad_n = [0]

            def emit_lad(sl):
                li = _lad_n[0]
                _lad_n[0] += 1
                nc.tensor.matmul(
                    lad_ps[0:1, 2 * li : 2 * li + 2], sl[:, 0:1], sl[:, 0:2],
                    start=True, stop=True,
                )

            # staged ladder schedule: (slot m, chunk ci) -> tensors to absorb
            LADDER_AT = {
                (8, 0): [qtd_sb[:, 7 * QS :], ktp_sb[:, :1024]],
                (8, 2): [va_sb[:, :1040]],
                (8, 4): [ktp_sb[:, 1024:]],
                (8, 6): [va_sb[:, 1040:]],
                (8, 7): ["mask"],
                (7, 0): [qtd_sb[:, 4 * QS : 7 * QS]],
                (4, 0): [qtd_sb[:, : 4 * QS]],
            }

            def emit_ladders(m, ci):
                for sl in LADDER_AT.get((m, ci), []):
                    if isinstance(sl, str):
                        nc.vector.tensor_copy(lad_dve[0:1, 0:8], mask_sb[0:1, 0:8])
                    else:
                        emit_lad(sl)

            # ---- flat pipeline state ----
            av_queue = []   # entries: ("av", slot_state, rec) / ("drain", slot_state)
            n_pending = [0]

            def emit_av(slot_state, rec):
                p_out = slot_state["p_out"]
                for j in range(rec["csize"]):
                    t = rec["base"] + j
                    slot_state["issued"] += 1
                    nc.tensor.matmul(
                        p_out,
                        va_sb[:, 65 * t : 65 * t + 65],
                        rec["rhs"][j],
                        start=(slot_state["issued"] == 1),
                        stop=(slot_state["issued"] == slot_state["T"]),
                    )

            def emit_drain(slot_state):
                o_sb = outs.tile([65, QS], F32, tag="drain")
                nc.vector.tensor_copy(o_sb, slot_state["p_out"])
                qlo = slot_state["qlo"]
                eng = nc.sync if (qlo // QS) % 2 == 0 else nc.gpsimd
                eng.dma_start(out=ot[:, qlo : qlo + QS], in_=o_sb)

            def pump(limit):
                while av_queue and (
                    n_pending[0] > limit or av_queue[0][0] == "drain"
                ):
                    kind, slot_state, *rest = av_queue.pop(0)
                    if kind == "drain":
                        emit_drain(slot_state)
                    else:
                        emit_av(slot_state, rest[0])
                        n_pending[0] -= 1

            # slots descending: deep pipelines first, 1-chunk slot last
            for m in range(NSLOTS, 0, -1):
                qlo = (m - 1) * QS
                T = 4 * m
                p_out = pso.tile([65, QS], F32, tag="avout", bufs=2)
                slot_state = {
                    "p_out": p_out,
                    "qlo": qlo,
                    "T": T,
                    "issued": 0,
                }
                deferred = []
                base = 0
                for ci, (csize, kind) in enumerate(_chunk_plan(m)):
                    emit_ladders(m, ci)
                    perm = PERM6 if csize == 6 else PERM4
                    p_sc = ps.tile([128, csize * QS], F32, tag="scores", bufs=3)
                    for j in range(csize):
                        t = base + j
                        h = t % 2
                        u = t // 2
                        nc.tensor.matmul(
                            p_sc[:, perm[j] * QS : perm[j] * QS + QS],
                            ktp_sb[64 * h : 64 * h + 64, 128 * u : 128 * u + 128],
                            qtd_sb[64 * h : 64 * h + 64, qlo : qlo + QS],
                            start=True,
                            stop=True,
                        )
                    if kind == "dve":
                        # y = ((s*(PA/128)+PB)^2 + PC)^2 ~ exp(s/64)
                        t_sb = work.tile([128, csize * QS], MBF16, tag="dvt", bufs=2)
                        nc.vector.tensor_scalar(
                            out=t_sb, in0=p_sc,
                            scalar1=float(PA / 128.0), scalar2=float(PB),
                            op0=MUL, op1=ADD,
                        )
                        v_sb = work.tile([128, csize * QS], MBF16, tag="dvv", bufs=2)
                        nc.vector.tensor_mul(v_sb, t_sb, t_sb)
                        w_sb = work.tile([128, csize * QS], MBF16, tag="dvw", bufs=2)
                        nc.vector.tensor_scalar_add(w_sb, v_sb, float(PC))
                        y_sb = work.tile([128, csize * QS], MBF16, tag="dvy", bufs=3)
                        nc.vector.tensor_mul(y_sb, w_sb, w_sb)
                        rhs = [
                            y_sb[:, perm[j] * QS : perm[j] * QS + QS]
                            for j in range(csize)
                        ]
                        deferred.append({"csize": csize, "base": base, "rhs": rhs})
                    else:
                        e_sb = work.tile([128, csize * QS], MBF16, tag="expT")
                        nc.scalar.activation(e_sb, p_sc, EXPF, scale=1.0 / 64.0)
                        if kind == "mask":
                            mk_sb = work.tile([128, 4 * QS], MBF16, tag="mko", bufs=2)
                            nc.vector.tensor_mul(mk_sb, e_sb, mask_sb)
                            src = mk_sb
                        else:
                            src = e_sb
                        rhs = [
                            src[:, perm[j] * QS : perm[j] * QS + QS]
                            for j in range(csize)
                        ]
                        av_queue.append(
                            ("av", slot_state, {"csize": csize, "base": base, "rhs": rhs})
                        )
                        n_pending[0] += 1
                    base += csize
                    pump(LAG)
                for rec in deferred:
                    av_queue.append(("av", slot_state, rec))
                    n_pending[0] += 1
                av_queue.append(("drain", slot_state))
                pump(LAG)
            pump(-1)

    nc.compile()
    return nc


def _get_compiled():
    global _COMPILED
    if _COMPILED is None:
        _COMPILED = _build_program()
    return _COMPILED


def _make_masks(half):
    ki = np.arange(KT_TILE)[:, None]
    qj = np.arange(QS)[None, :]
    d_a = (qj >= ki).astype(np.float32)
    d_b = (qj >= ki + 128).astype(np.float32)
    ones = np.ones((KT_TILE, QS), np.float32)
    zeros = np.zeros((KT_TILE, QS), np.float32)
    blocks = [ones, ones, d_a, d_b] if half == 0 else [d_a, d_b, zeros, zeros]
    # permute columns so tile j of the last chunk lands at PERM4[j]*QS,
    # matching the PSUM bank permutation of the QK outputs
    out = np.empty((KT_TILE, 4 * QS), np.float32)
    for j in range(4):
        out[:, PERM4[j] * QS : PERM4[j] * QS + QS] = blocks[j]
    return out


def make_in_maps(Q, K, V):
    """Pack full fp32 Q,K,V [B,S,D] into 8 per-core input dicts."""
    in_maps = []
    for c in range(NCORES):
        b = c % 4
        half = c // 4
        # KT packed: k-tile t -> partition half t%2, cols 128*(t//2)
        kt = np.ascontiguousarray(K[b].T)  # [64, 4096]
        ktp = np.empty((128, 16 * 128), np.float32)
        for t in range(NKT):
            h, u = t % 2, t // 2
            ktp[64 * h : 64 * h + 64, 128 * u : 128 * u + 128] = kt[
                :, 128 * t : 128 * t + 128
            ]
        # Q slots (duplicated into both partition halves)
        qrows = np.concatenate(
            [Q[b, 256 * (2 * m - 1 - half) : 256 * (2 * m - 1 - half) + 256] for m in range(1, 9)],
            axis=0,
        )  # [2048, 64]
        qt = np.ascontiguousarray(qrows.T)  # [64, 2048]
        qtd = np.concatenate([qt, qt], axis=0)  # [128, 2048]
        # V augmented with ones column, tiles side by side
        va = np.empty((128, NKT * 65), np.float32)
        for t in range(NKT):
            va[:, 65 * t : 65 * t + 64] = V[b, 128 * t : 128 * t + 128, :]
            va[:, 65 * t + 64] = 1.0
        in_maps.append(
            {
                "ktp": ktp.astype(BF16),
                "qtd": qtd.astype(BF16),
                "va": va.astype(BF16),
                "mask": _make_masks(half).astype(BF16),
            }
        )
    return in_maps


def unpack_outputs(results):
    """Combine 8 per-core OT [65, 2048] fp32 into full output [B,S,D]."""
    out = np.empty((B, S, D), np.float32)
    for c in range(NCORES):
        b = c % 4
        half = c // 4
        otc = results[c]["ot"]  # [65, 2048]
        for m in range(1, 9):
            j = 2 * m - 1 - half
            sl = otc[:, 256 * (m - 1) : 256 * m]  # [65, 256]
            out[b, 256 * j : 256 * j + 256, :] = (sl[:64] / sl[64:65]).T
    return out


def run_on_hw(in_maps, trace=False, trace_cores=None):
    from concourse.bass_utils import run_bass_kernel_spmd

    nc = _get_compiled()
    return run_bass_kernel_spmd(
        nc, in_maps, core_ids=list(range(NCORES)), trace=trace, trace_cores=trace_cores
    )


def kernel(Q, K, V):
    Q = np.asarray(Q, np.float32)
    K = np.asarray(K, np.float32)
    V = np.asarray(V, np.float32)
    res = run_on_hw(make_in_maps(Q, K, V), trace=False)
    return unpack_outputs(res.results)
